# revision 6
# baseline (speedup 1.0000x reference)
"""Trainium2 Bass kernel for ChaoticAttentionLayer.

Math (reference):
    q = r_s * sig(zq) * (1 - sig(zq)),  zq = query @ Wq.T + bq,  r_s = 4*sigmoid(r)
    k likewise, v = value @ Wv.T + bv
    out = softmax(q k^T / 8) v @ Wo.T + bo   (per head, D=64)

Device decomposition:
    g = sig*(1-sig); scores = (r_s^2/8) * g(zq) . g(zk); the r_s^2/8 factor is
    folded into the Exp activation's scale. Scores are bounded in [0, 8] for
    any r, so softmax runs max-free: exp(scores) directly.

Sharding: 8 cores = 4 batches x 2 head-groups (4 heads each). Each core
computes partial out[b] = attn_hg @ Wo[:, hg].T; host sums the two partials
per batch and adds bo.

Structure (per core, all bf16 on matmul paths):
  - Scores are computed transposed, S^T[s_block, t], two heads per Exp call
    (row-tiled 64-contraction matmuls run concurrently on the PE).
  - attn@V uses V as the stationary operand: av[d, t] = V'^T ex, one
    N=512 matmul per (head, s_block). V' is padded per head to 128 cols:
    even head [V | 1 | 0*63], odd head [0*63 | 1 | V], so the even head's
    numerator lands at PSUM partitions 0..63 and the odd head's at 64..127.
    The softmax denominator comes out at row 64 / 63 respectively.
  - Normalization: 1/den via reciprocal_approx_fast on the den row, DMA
    partition-broadcast of the reciprocal row, then one tensor_mul per head
    writes the normalized, already-transposed attention output aTn[128, t]
    (both heads of a pair stacked) -- no PE transposes needed.
  - Out-projection: full K=128 contraction lhsT=aTn, accumulated over the
    two head pairs; final scale+copy and DMA out.
"""

import numpy as np
import ml_dtypes
from contextlib import ExitStack

try:
    import concourse.bass as bass
except ImportError:  # pragma: no cover
    import sys

    sys.path.insert(0, "/opt/trn_rl_repo")
    import concourse.bass as bass

import concourse.bacc as bacc
import concourse.tile as tile
from concourse import mybir
from concourse.bass_utils import run_bass_kernel_spmd

F32 = mybir.dt.float32
BF16 = mybir.dt.bfloat16
AF = mybir.ActivationFunctionType
BF16NP = ml_dtypes.bfloat16

B, T, S, E, H = 4, 2048, 2048, 512, 8
D = E // H           # 64 head dim
HG = 2               # head-groups per batch (cores per batch)
HPG = H // HG        # 4 heads per group
EG = HPG * D         # 256 dims per head group
NCORES = 8
P = 128              # partitions
TCH = 512            # t-chunk (psum free dim)
NSB = S // P         # 16 s-blocks
NKT = E // P         # 4 contraction tiles of 128
NTC = T // TCH       # 4 t-chunks


def _build():
    nc = bacc.Bacc("TRN2", target_bir_lowering=False, debug=False,
                   num_devices=NCORES)

    xqT = nc.dram_tensor("xqT", [E, T], BF16, kind="ExternalInput")
    xkT = nc.dram_tensor("xkT", [E, S], BF16, kind="ExternalInput")
    xvT = nc.dram_tensor("xvT", [E + 1, S], BF16, kind="ExternalInput")
    wqT = nc.dram_tensor("wqT", [E, EG], BF16, kind="ExternalInput")
    wkT = nc.dram_tensor("wkT", [E, EG], BF16, kind="ExternalInput")
    wvT = nc.dram_tensor("wvT", [E + 1, EG], BF16, kind="ExternalInput")
    woT = nc.dram_tensor("woT", [EG, E], BF16, kind="ExternalInput")
    bq = nc.dram_tensor("bq", [EG, 1], F32, kind="ExternalInput")
    bk = nc.dram_tensor("bk", [EG, 1], F32, kind="ExternalInput")
    cexp = nc.dram_tensor("cexp", [1, 1], F32, kind="ExternalInput")
    out = nc.dram_tensor("out", [T, E], F32, kind="ExternalOutput")

    with tile.TileContext(nc) as tc, ExitStack() as ctx:
        persist = ctx.enter_context(tc.tile_pool(name="persist", bufs=1))

        # --- persistent SBUF state ---
        # K-projection inputs first: they gate the whole pipeline.
        wk_sb = []
        bk_sb = []
        for kt in range(NKT):
            tk = persist.tile([P, EG], BF16, tag=f"wk{kt}")
            nc.sync.dma_start(out=tk, in_=wkT[kt * P:(kt + 1) * P, :])
            wk_sb.append(tk)
        for c in range(EG // P):
            tb2 = persist.tile([P, 1], F32, tag=f"bk{c}")
            nc.sync.dma_start(out=tb2, in_=bk[c * P:(c + 1) * P, :])
            bk_sb.append(tb2)
        wv_sb = []
        for kt in range(NKT):
            tv = persist.tile([P, EG], BF16, tag=f"wv{kt}")
            nc.sync.dma_start(out=tv, in_=wvT[kt * P:(kt + 1) * P, :])
            wv_sb.append(tv)
        wv4_sb = persist.tile([1, EG], BF16, tag="wv4")
        nc.sync.dma_start(out=wv4_sb, in_=wvT[E:E + 1, :])
        wq_sb = []
        bq_sb = []
        for kt in range(NKT):
            tq = persist.tile([P, EG], BF16, tag=f"wq{kt}")
            nc.sync.dma_start(out=tq, in_=wqT[kt * P:(kt + 1) * P, :])
            wq_sb.append(tq)
        for c in range(EG // P):
            tb_ = persist.tile([P, 1], F32, tag=f"bq{c}")
            nc.sync.dma_start(out=tb_, in_=bq[c * P:(c + 1) * P, :])
            bq_sb.append(tb_)

        ones_sb = persist.tile([1, S], BF16, tag="ones")
        nc.sync.dma_start(out=ones_sb, in_=xvT[E:E + 1, :])

        cexp_sb = persist.tile([P, 1], F32, tag="cexp")
        cap = cexp[:, :]
        nc.sync.dma_start(
            out=cexp_sb,
            in_=bass.AP(tensor=cap.tensor, offset=cap.offset, ap=[[0, P], [1, 1]]),
        )

        # out-proj weights are only needed later.
        wo_sb = []
        for kb in range(EG // P):
            to = persist.tile([P, E], BF16, tag=f"wo{kb}")
            nc.sync.dma_start(out=to, in_=woT[kb * P:(kb + 1) * P, :])
            wo_sb.append(to)

        # projected tensors, resident for the whole kernel; chunked into
        # [P, TCH] column tiles so consumers unblock per-chunk.
        QT_sb = [[persist.tile([P, TCH], BF16, tag=f"qt{c}_{q}",
                               name=f"qt{c}_{q}") for q in range(NTC)]
                 for c in range(EG // P)]
        KT_sb = [[persist.tile([P, TCH], BF16, tag=f"kt{c}_{q}",
                               name=f"ktile{c}_{q}") for q in range(NTC)]
                 for c in range(EG // P)]
        # V', padded to 128 cols per head:
        #   even head h: [V_h (64) | ones (1) | zeros (63)]
        #   odd  head h: [zeros (63) | ones (1) | V_h (64)]
        V2_sb = [persist.tile([P, HPG, P], BF16, tag=f"v{sc}", name=f"v{sc}")
                 for sc in range(NSB)]

        # --- projections (x inputs resident in SBUF as big tiles) ---
        xk_sb = [[None] * NTC for _ in range(NKT)]
        xv_sb = [[None] * NTC for _ in range(NKT)]
        xq_sb = [[None] * NTC for _ in range(NKT)]
        for q in range(NTC):
            for kt in range(NKT):
                xk_t = persist.tile([P, TCH], BF16, tag=f"xk{kt}_{q}",
                                    name=f"xk{kt}_{q}")
                nc.sync.dma_start(
                    out=xk_t,
                    in_=xkT[kt * P:(kt + 1) * P, q * TCH:(q + 1) * TCH])
                xk_sb[kt][q] = xk_t
            for kt in range(NKT):
                xq_t = persist.tile([P, TCH], BF16, tag=f"xq{kt}_{q}",
                                    name=f"xq{kt}_{q}")
                nc.sync.dma_start(
                    out=xq_t,
                    in_=xqT[kt * P:(kt + 1) * P, q * TCH:(q + 1) * TCH])
                xq_sb[kt][q] = xq_t
            for kt in range(NKT):
                xv_t = persist.tile([P, TCH], BF16, tag=f"xvr{kt}_{q}",
                                    name=f"xvr{kt}_{q}")
                nc.sync.dma_start(
                    out=xv_t,
                    in_=xvT[kt * P:(kt + 1) * P, q * TCH:(q + 1) * TCH])
                xv_sb[kt][q] = xv_t

        def qk_proj_chunk(pool, x_sb, w_sb, b_sb, out_tiles, c, tcq, sig):
            ps = pool.tile([P, TCH], F32, tag="ps", name=f"ps_{c}_{tcq}")
            for kt in range(NKT):
                nc.tensor.matmul(
                    ps, w_sb[kt][:, c * P:(c + 1) * P],
                    x_sb[kt][tcq],
                    start=(kt == 0), stop=(kt == NKT - 1))
            # sig'(z) = (1 - tanh^2(z/2)) / 4 -- tanh shares ACT's exp
            # table set, so the whole kernel needs one ACT_TABLE_LOAD.
            y = sig.tile([P, TCH], F32, tag="y", name=f"y_{c}_{tcq}")
            nc.scalar.activation(y, ps, AF.Tanh, bias=b_sb[c], scale=0.5)
            y2 = sig.tile([P, TCH], F32, tag="y2", name=f"y2_{c}_{tcq}")
            nc.vector.tensor_mul(y2, y, y)
            nc.vector.tensor_scalar(out_tiles[c][tcq], y2, -0.25, 0.25,
                                    mybir.AluOpType.mult,
                                    mybir.AluOpType.add)

        def proj_group(tcq, psp, sig):
            for c in range(EG // P):
                qk_proj_chunk(psp, xk_sb, wk_sb, bk_sb, KT_sb, c, tcq, sig)
            for c in range(EG // P):
                qk_proj_chunk(psp, xq_sb, wq_sb, bq_sb, QT_sb, c, tcq, sig)
            for sci in range(4):
                sc = tcq * 4 + sci
                ps = psp.tile([P, TCH], F32, tag="ps", name=f"psv_{sc}")
                for kt in range(NKT):
                    nc.tensor.matmul(ps[:, 0:EG],
                                     xv_sb[kt][tcq][:, sci * P:(sci + 1) * P],
                                     wv_sb[kt], start=(kt == 0), stop=False)
                nc.tensor.matmul(ps[:, 0:EG], ones_sb[:, sc * P:(sc + 1) * P],
                                 wv4_sb, start=False, stop=True)
                v2 = V2_sb[sc]
                psv = ps[:, 0:EG].rearrange("p (h d) -> p h d", h=HPG)
                # even heads: V at cols 0..63, ones at 64, zeros above
                nc.vector.tensor_copy(v2[:, 0::2, 0:D], psv[:, 0::2, :])
                nc.vector.memset(v2[:, 0::2, D:D + 1], 1.0)
                nc.vector.memset(v2[:, 0::2, D + 1:P], 0.0)
                # odd heads: ones at 0, zeros at 1..63, V at cols 64..127
                nc.vector.tensor_copy(v2[:, 1::2, D:P], psv[:, 1::2, :])
                nc.vector.memset(v2[:, 1::2, 0:1], 1.0)
                nc.vector.memset(v2[:, 1::2, 1:D], 0.0)

        # --- attention + out-projection ---
        expp = ctx.enter_context(tc.tile_pool(name="expp", bufs=3))
        aTnp = ctx.enter_context(tc.tile_pool(name="aTnp", bufs=2))
        denp = ctx.enter_context(tc.tile_pool(name="denp", bufs=2))
        rdbp = ctx.enter_context(tc.tile_pool(name="rdbp", bufs=2))
        outp = ctx.enter_context(tc.tile_pool(name="outp", bufs=3))

        def attention_tci(tci, pss, pmix, pending):
            aTn = [aTnp.tile([P, TCH], BF16, tag=f"aTn{hp}",
                             name=f"aTn_{tci}_{hp}") for hp in range(2)]
            for hp in range(2):
                ph = tci * 2 + hp
                avE = pmix.tile([P, TCH], F32, tag=f"av{(2 * ph) % 3}",
                                name=f"avE_{tci}_{hp}")
                avO = pmix.tile([P, TCH], F32, tag=f"av{(2 * ph + 1) % 3}",
                                name=f"avO_{tci}_{hp}")
                avs = [avE, avO]
                exs = [None] * NSB

                def emit_av(sb):
                    for hi in range(2):
                        h = 2 * hp + hi
                        nc.tensor.matmul(
                            avs[hi],
                            V2_sb[sb][:, h, :],
                            exs[sb][:, hi * TCH:(hi + 1) * TCH],
                            start=(sb == 0), stop=(sb == NSB - 1),
                            skip_group_check=(sb != 0))

                for sb in range(NSB):
                    ps = pss.tile([P, 2 * TCH], F32, tag="ps",
                                  name=f"ps_{tci}_{hp}_{sb}")
                    for hi in range(2):
                        h = 2 * hp + hi
                        off = hi * D
                        nc.tensor.matmul(
                            ps[:, hi * TCH:(hi + 1) * TCH],
                            KT_sb[hp][sb // 4][off:off + D,
                                               (sb % 4) * P:(sb % 4 + 1) * P],
                            QT_sb[hp][tci][off:off + D, :],
                            start=True, stop=True,
                            tile_position=(off, 0))
                    ex = expp.tile([P, 2 * TCH], BF16, tag="ex",
                                   name=f"ex_{tci}_{hp}_{sb}")
                    nc.scalar.activation(ex, ps, AF.Exp, scale=cexp_sb)
                    exs[sb] = ex
                    if sb == 3 and hp == 0 and pending is not None:
                        pending()
                    if sb >= 1:
                        emit_av(sb - 1)
                emit_av(NSB - 1)

                # normalization: den rows -> SBUF (same partition), shift the
                # even-head den to partition 0 via DMA, broadcast both rows,
                # one reciprocal, then scale -> aTn[hp]
                dencp = denp.tile([P, TCH], F32, tag="dencp",
                                  name=f"dencp_{tci}_{hp}")
                nc.vector.tensor_copy(dencp[D:D + 1, :], avE[D:D + 1, :])
                nc.vector.tensor_copy(dencp[0:1, :], avO[0:1, :])
                den0 = denp.tile([1, TCH], F32, tag="den0",
                                 name=f"den0_{tci}_{hp}")
                nc.sync.dma_start(out=den0, in_=dencp[D:D + 1, :])
                denb = rdbp.tile([P, TCH], F32, tag="denb",
                                 name=f"denb_{tci}_{hp}")
                e0 = den0[0:1, :]
                e1 = dencp[0:1, :]
                nc.sync.dma_start(
                    out=denb[0:D, :],
                    in_=bass.AP(tensor=e0.tensor, offset=e0.offset,
                                ap=[[1, 1], [0, D], [1, TCH]]))
                nc.sync.dma_start(
                    out=denb[D:P, :],
                    in_=bass.AP(tensor=e1.tensor, offset=e1.offset,
                                ap=[[1, 1], [0, D], [1, TCH]]))
                rdb = rdbp.tile([P, TCH], F32, tag="rdb",
                                name=f"rdb_{tci}_{hp}")
                nc.vector.reciprocal_approx_fast(out=rdb, in_=denb)
                nc.vector.tensor_mul(aTn[hp][0:D, :], avE[0:D, :],
                                     rdb[0:D, :])
                nc.vector.tensor_mul(aTn[hp][D:P, :], avO[D:P, :],
                                     rdb[D:P, :])

            def emit_outproj():
                for ts in range(4):
                    pf = pmix.tile([P, E], F32, tag="pf",
                                   name=f"pf_{tci}_{ts}")
                    for hp2 in range(2):
                        nc.tensor.matmul(pf,
                                         aTn[hp2][:, ts * P:(ts + 1) * P],
                                         wo_sb[hp2],
                                         start=(hp2 == 0), stop=(hp2 == 1))
                    ot = outp.tile([P, E], F32, tag="ot",
                                   name=f"ot_{tci}_{ts}")
                    nc.vector.tensor_copy(ot, pf)
                    row = (tci * 4 + ts) * P
                    nc.sync.dma_start(out=out[row:row + P, :], in_=ot)

            return emit_outproj

        with ExitStack() as cp:
            psp = cp.enter_context(
                tc.tile_pool(name="psp", bufs=2, space="PSUM"))
            sig = cp.enter_context(tc.tile_pool(name="sig", bufs=4))
            for tcq in range(NTC):
                proj_group(tcq, psp, sig)
        with ExitStack() as c4:
            pss = c4.enter_context(
                tc.tile_pool(name="pss", bufs=2, space="PSUM"))
            pmix = c4.enter_context(
                tc.tile_pool(name="pmix", bufs=1, space="PSUM"))
            pending = None
            for tcq in range(NTC):
                pending = attention_tci(tcq, pss, pmix, pending)
            pending()

    nc.compile()
    return nc


_NC = None
_LAST_IN_MAPS = None


def _get_nc():
    global _NC
    if _NC is None:
        _NC = _build()
    return _NC


def kernel(**inputs):
    query = np.asarray(inputs["query"], np.float32)
    key_ = np.asarray(inputs["key_"] if "key_" in inputs else inputs["key"],
                      np.float32)
    value = np.asarray(inputs["value"], np.float32)
    Wq = np.asarray(inputs["Wq"], np.float32)
    bq = np.asarray(inputs["bq"], np.float32)
    Wk = np.asarray(inputs["Wk"], np.float32)
    bk = np.asarray(inputs["bk"], np.float32)
    Wv = np.asarray(inputs["Wv"], np.float32)
    bv = np.asarray(inputs["bv"], np.float32)
    Wo = np.asarray(inputs["Wo"], np.float32)
    bo = np.asarray(inputs["bo"], np.float32)
    r = float(np.asarray(inputs["r"]).reshape(-1)[0])

    r_s = 4.0 / (1.0 + np.exp(-np.float64(r)))
    c = np.float32(r_s * r_s / 8.0)

    WqT = Wq.T.astype(BF16NP)
    WkT = Wk.T.astype(BF16NP)
    WoT = Wo.T.astype(BF16NP)
    WvTa = np.concatenate([Wv.T, bv[None, :]], axis=0).astype(BF16NP)

    in_maps = []
    for b in range(B):
        xqT = np.ascontiguousarray(query[b].T).astype(BF16NP)
        xkT = np.ascontiguousarray(key_[b].T).astype(BF16NP)
        xvT = np.concatenate(
            [np.ascontiguousarray(value[b].T), np.ones((1, S), np.float32)],
            axis=0).astype(BF16NP)
        for g in range(HG):
            cols = slice(g * EG, (g + 1) * EG)
            in_maps.append(dict(
                xqT=xqT, xkT=xkT, xvT=xvT,
                wqT=np.ascontiguousarray(WqT[:, cols]),
                wkT=np.ascontiguousarray(WkT[:, cols]),
                wvT=np.ascontiguousarray(WvTa[:, cols]),
                woT=np.ascontiguousarray(WoT[cols, :]),
                bq=np.ascontiguousarray(0.5 * bq[cols, None]),
                bk=np.ascontiguousarray(0.5 * bk[cols, None]),
                cexp=np.array([[c]], np.float32),
            ))

    global _LAST_IN_MAPS
    _LAST_IN_MAPS = in_maps
    res = run_bass_kernel_spmd(_get_nc(), in_maps, core_ids=list(range(NCORES)))
    out = np.empty((B, T, E), np.float32)
    for b in range(B):
        out[b] = res.results[HG * b]["out"]
        for g in range(1, HG):
            out[b] += res.results[HG * b + g]["out"]
        out[b] += bo[None, :]
    return out


# revision 8
# speedup vs baseline: 1.1084x; 1.1084x over previous
"""Trainium2 Bass kernel for ChaoticAttentionLayer.

Math (reference):
    q = r_s * sig(zq) * (1 - sig(zq)),  zq = query @ Wq.T + bq,  r_s = 4*sigmoid(r)
    k likewise, v = value @ Wv.T + bv
    out = softmax(q k^T / 8) v @ Wo.T + bo   (per head, D=64)

Device decomposition:
    g = sig*(1-sig); scores = (r_s^2/8) * g(zq) . g(zk); the r_s^2/8 factor is
    folded into the Exp activation's scale. Scores are bounded in [0, 8] for
    any r, so softmax runs max-free: exp(scores) directly.

Sharding: 8 cores = 4 batches x 2 head-groups (4 heads each). Each core
computes partial out[b] = attn_hg @ Wo[:, hg].T; host sums the two partials
per batch and adds bo.

Structure (per core, all bf16 on matmul paths):
  - Scores are computed transposed, S^T[s_block, t], two heads per Exp call
    (row-tiled 64-contraction matmuls run concurrently on the PE).
  - attn@V uses V as the stationary operand: av[d, t] = V'^T ex, one
    N=512 matmul per (head, s_block). V' is padded per head to 128 cols:
    even head [V | 1 | 0*63], odd head [0*63 | 1 | V], so the even head's
    numerator lands at PSUM partitions 0..63 and the odd head's at 64..127.
    The softmax denominator comes out at row 64 / 63 respectively.
  - Normalization: 1/den via reciprocal_approx_fast on the den row, DMA
    partition-broadcast of the reciprocal row, then one tensor_mul per head
    writes the normalized, already-transposed attention output aTn[128, t]
    (both heads of a pair stacked) -- no PE transposes needed.
  - Out-projection: full K=128 contraction lhsT=aTn, accumulated over the
    two head pairs; final scale+copy and DMA out.
"""

import numpy as np
import ml_dtypes
from contextlib import ExitStack

try:
    import concourse.bass as bass
except ImportError:  # pragma: no cover
    import sys

    sys.path.insert(0, "/opt/trn_rl_repo")
    import concourse.bass as bass

import concourse.bacc as bacc
import concourse.tile as tile
from concourse import mybir
from concourse.bass_utils import run_bass_kernel_spmd

F32 = mybir.dt.float32
BF16 = mybir.dt.bfloat16
AF = mybir.ActivationFunctionType
BF16NP = ml_dtypes.bfloat16

B, T, S, E, H = 4, 2048, 2048, 512, 8
D = E // H           # 64 head dim
HG = 2               # head-groups per batch (cores per batch)
HPG = H // HG        # 4 heads per group
EG = HPG * D         # 256 dims per head group
NCORES = 8
P = 128              # partitions
TCH = 512            # t-chunk (psum free dim)
NSB = S // P         # 16 s-blocks
NKT = E // P         # 4 contraction tiles of 128
NTC = T // TCH       # 4 t-chunks


def _build():
    nc = bacc.Bacc("TRN2", target_bir_lowering=False, debug=False,
                   num_devices=NCORES)

    xqT = nc.dram_tensor("xqT", [E, T], BF16, kind="ExternalInput")
    xkT = nc.dram_tensor("xkT", [E, S], BF16, kind="ExternalInput")
    xvT = nc.dram_tensor("xvT", [E + 1, S], BF16, kind="ExternalInput")
    wqT = nc.dram_tensor("wqT", [E, EG], BF16, kind="ExternalInput")
    wkT = nc.dram_tensor("wkT", [E, EG], BF16, kind="ExternalInput")
    wvT = nc.dram_tensor("wvT", [E + 1, EG], BF16, kind="ExternalInput")
    woT = nc.dram_tensor("woT", [EG, E], BF16, kind="ExternalInput")
    bq = nc.dram_tensor("bq", [EG, 1], F32, kind="ExternalInput")
    bk = nc.dram_tensor("bk", [EG, 1], F32, kind="ExternalInput")
    cexp = nc.dram_tensor("cexp", [1, 1], F32, kind="ExternalInput")
    out = nc.dram_tensor("out", [T, E], F32, kind="ExternalOutput")

    with tile.TileContext(nc) as tc, ExitStack() as ctx:
        persist = ctx.enter_context(tc.tile_pool(name="persist", bufs=1))

        # --- persistent SBUF state ---
        # K-projection inputs first: they gate the whole pipeline.
        wk_sb = []
        bk_sb = []
        for kt in range(NKT):
            tk = persist.tile([P, EG], BF16, tag=f"wk{kt}")
            nc.sync.dma_start(out=tk, in_=wkT[kt * P:(kt + 1) * P, :])
            wk_sb.append(tk)
        for c in range(EG // P):
            tb2 = persist.tile([P, 1], F32, tag=f"bk{c}")
            nc.sync.dma_start(out=tb2, in_=bk[c * P:(c + 1) * P, :])
            bk_sb.append(tb2)
        # projected tensors, resident for the whole kernel; chunked into
        # [P, TCH] column tiles so consumers unblock per-chunk.
        QT_sb = [[persist.tile([P, TCH], BF16, tag=f"qt{c}_{q}",
                               name=f"qt{c}_{q}") for q in range(NTC)]
                 for c in range(EG // P)]
        KT_sb = [[persist.tile([P, TCH], BF16, tag=f"kt{c}_{q}",
                               name=f"ktile{c}_{q}") for q in range(NTC)]
                 for c in range(EG // P)]
        # V', padded to 128 cols per head:
        #   even head h: [V_h (64) | ones (1) | zeros (63)]
        #   odd  head h: [ones (1) | zeros (63) | V_h (64)]
        V2_sb = [persist.tile([P, HPG, P], BF16, tag=f"v{sc}", name=f"v{sc}")
                 for sc in range(NSB)]

        # --- x inputs + remaining weights, in consumption order ---
        xk_sb = [[None] * NTC for _ in range(NKT)]
        xv_sb = [[None] * NTC for _ in range(NKT)]
        xq_sb = [[None] * NTC for _ in range(NKT)]

        def load_x(dst, src, kt, q):
            t = persist.tile([P, TCH], BF16, tag=f"{dst}{kt}_{q}",
                             name=f"{dst}{kt}_{q}")
            nc.sync.dma_start(
                out=t, in_=src[kt * P:(kt + 1) * P, q * TCH:(q + 1) * TCH])
            return t

        for kt in range(NKT):
            xk_sb[kt][0] = load_x("xk", xkT, kt, 0)

        wq_sb = []
        bq_sb = []
        for kt in range(NKT):
            tq = persist.tile([P, EG], BF16, tag=f"wq{kt}")
            nc.sync.dma_start(out=tq, in_=wqT[kt * P:(kt + 1) * P, :])
            wq_sb.append(tq)
        for c in range(EG // P):
            tb_ = persist.tile([P, 1], F32, tag=f"bq{c}")
            nc.sync.dma_start(out=tb_, in_=bq[c * P:(c + 1) * P, :])
            bq_sb.append(tb_)
        for kt in range(NKT):
            xq_sb[kt][0] = load_x("xq", xqT, kt, 0)

        wv_sb = []
        for kt in range(NKT):
            tv = persist.tile([P, EG], BF16, tag=f"wv{kt}")
            nc.sync.dma_start(out=tv, in_=wvT[kt * P:(kt + 1) * P, :])
            wv_sb.append(tv)
        wv4_sb = persist.tile([1, EG], BF16, tag="wv4")
        nc.sync.dma_start(out=wv4_sb, in_=wvT[E:E + 1, :])
        ones_sb = persist.tile([1, S], BF16, tag="ones")
        nc.sync.dma_start(out=ones_sb, in_=xvT[E:E + 1, :])
        for kt in range(NKT):
            xv_sb[kt][0] = load_x("xvr", xvT, kt, 0)

        cexp_sb = persist.tile([P, 1], F32, tag="cexp")
        cap = cexp[:, :]
        nc.sync.dma_start(
            out=cexp_sb,
            in_=bass.AP(tensor=cap.tensor, offset=cap.offset, ap=[[0, P], [1, 1]]),
        )

        for q in range(1, NTC):
            for kt in range(NKT):
                xk_sb[kt][q] = load_x("xk", xkT, kt, q)
            for kt in range(NKT):
                xq_sb[kt][q] = load_x("xq", xqT, kt, q)
            for kt in range(NKT):
                xv_sb[kt][q] = load_x("xvr", xvT, kt, q)

        # out-proj weights are only needed much later.
        wo_sb = []
        for kb in range(EG // P):
            to = persist.tile([P, E], BF16, tag=f"wo{kb}")
            nc.sync.dma_start(out=to, in_=woT[kb * P:(kb + 1) * P, :])
            wo_sb.append(to)

        def qk_proj_chunk(pool, x_sb, w_sb, b_sb, out_tiles, c, tcq, sig):
            ps = pool.tile([P, TCH], F32, tag="ps", name=f"ps_{c}_{tcq}")
            for kt in range(NKT):
                nc.tensor.matmul(
                    ps, w_sb[kt][:, c * P:(c + 1) * P],
                    x_sb[kt][tcq],
                    start=(kt == 0), stop=(kt == NKT - 1))
            # sig'(z) = (1 - tanh^2(z/2)) / 4 -- tanh shares ACT's exp
            # table set, so the whole kernel needs one ACT_TABLE_LOAD.
            y = sig.tile([P, TCH], F32, tag="y", name=f"y_{c}_{tcq}")
            nc.scalar.activation(y, ps, AF.Tanh, bias=b_sb[c], scale=0.5)
            y2 = sig.tile([P, TCH], F32, tag="y2", name=f"y2_{c}_{tcq}")
            nc.vector.tensor_mul(y2, y, y)
            nc.vector.tensor_scalar(out_tiles[c][tcq], y2, -0.25, 0.25,
                                    mybir.AluOpType.mult,
                                    mybir.AluOpType.add)

        def proj_group(tcq, psp, sig):
            for c in range(EG // P):
                qk_proj_chunk(psp, xk_sb, wk_sb, bk_sb, KT_sb, c, tcq, sig)
            for c in range(EG // P):
                qk_proj_chunk(psp, xq_sb, wq_sb, bq_sb, QT_sb, c, tcq, sig)
            for sci in range(4):
                sc = tcq * 4 + sci
                ps = psp.tile([P, TCH], F32, tag="ps", name=f"psv_{sc}")
                for kt in range(NKT):
                    nc.tensor.matmul(ps[:, 0:EG],
                                     xv_sb[kt][tcq][:, sci * P:(sci + 1) * P],
                                     wv_sb[kt], start=(kt == 0), stop=False)
                nc.tensor.matmul(ps[:, 0:EG], ones_sb[:, sc * P:(sc + 1) * P],
                                 wv4_sb, start=False, stop=True)
                v2 = V2_sb[sc]
                psv = ps[:, 0:EG].rearrange("p (h d) -> p h d", h=HPG)
                # even heads: V at cols 0..63, ones at 64, zeros above
                nc.vector.tensor_copy(v2[:, 0::2, 0:D], psv[:, 0::2, :])
                nc.vector.memset(v2[:, 0::2, D:D + 1], 1.0)
                nc.vector.memset(v2[:, 0::2, D + 1:P], 0.0)
                # odd heads: ones at 0, zeros at 1..63, V at cols 64..127
                nc.vector.tensor_copy(v2[:, 1::2, D:P], psv[:, 1::2, :])
                nc.vector.memset(v2[:, 1::2, 0:1], 1.0)
                nc.vector.memset(v2[:, 1::2, 1:D], 0.0)

        # --- attention + out-projection ---
        expp = ctx.enter_context(tc.tile_pool(name="expp", bufs=3))
        aTnp = ctx.enter_context(tc.tile_pool(name="aTnp", bufs=2))
        denp = ctx.enter_context(tc.tile_pool(name="denp", bufs=2))
        rdbp = ctx.enter_context(tc.tile_pool(name="rdbp", bufs=2))
        outp = ctx.enter_context(tc.tile_pool(name="outp", bufs=3))

        def attention_tci(tci, pss, pmix, pending):
            aTn = [aTnp.tile([P, TCH], BF16, tag=f"aTn{hp}",
                             name=f"aTn_{tci}_{hp}") for hp in range(2)]
            for hp in range(2):
                ph = tci * 2 + hp
                avE = pmix.tile([P, TCH], F32, tag=f"av{(2 * ph) % 3}",
                                name=f"avE_{tci}_{hp}")
                avO = pmix.tile([P, TCH], F32, tag=f"av{(2 * ph + 1) % 3}",
                                name=f"avO_{tci}_{hp}")
                avs = [avE, avO]
                exs = [None] * NSB

                def emit_av(sb):
                    for hi in range(2):
                        h = 2 * hp + hi
                        nc.tensor.matmul(
                            avs[hi],
                            V2_sb[sb][:, h, :],
                            exs[sb][:, hi * TCH:(hi + 1) * TCH],
                            start=(sb == 0), stop=(sb == NSB - 1),
                            skip_group_check=(sb != 0))

                for sb in range(NSB):
                    ps = pss.tile([P, 2 * TCH], F32, tag="ps",
                                  name=f"ps_{tci}_{hp}_{sb}")
                    for hi in range(2):
                        h = 2 * hp + hi
                        off = hi * D
                        nc.tensor.matmul(
                            ps[:, hi * TCH:(hi + 1) * TCH],
                            KT_sb[hp][sb // 4][off:off + D,
                                               (sb % 4) * P:(sb % 4 + 1) * P],
                            QT_sb[hp][tci][off:off + D, :],
                            start=True, stop=True,
                            tile_position=(off, 0))
                    ex = expp.tile([P, 2 * TCH], BF16, tag="ex",
                                   name=f"ex_{tci}_{hp}_{sb}")
                    nc.scalar.activation(ex, ps, AF.Exp, scale=cexp_sb)
                    exs[sb] = ex
                    if sb == 3 and hp == 0 and pending is not None:
                        pending()
                    if sb >= 1:
                        emit_av(sb - 1)
                emit_av(NSB - 1)

                # Copy both av banks to SBUF immediately so the PSUM banks
                # free up for the next phase; the (DMA-latency-heavy) norm
                # chain then runs entirely off the PE critical path.
                avsE = denp.tile([P, TCH], F32, tag="avsE",
                                 name=f"avsE_{tci}_{hp}")
                avsO = denp.tile([P, TCH], F32, tag="avsO",
                                 name=f"avsO_{tci}_{hp}")
                nc.vector.tensor_copy(avsE[0:D + 1, :], avE[0:D + 1, :])
                nc.vector.tensor_copy(avsO, avO)
                # normalization: shift the even-head den row to partition 0
                # via DMA, broadcast both den rows, one reciprocal, scale.
                den0 = denp.tile([1, TCH], F32, tag="den0",
                                 name=f"den0_{tci}_{hp}")
                nc.sync.dma_start(out=den0, in_=avsE[D:D + 1, :])
                denb = rdbp.tile([P, TCH], F32, tag="denb",
                                 name=f"denb_{tci}_{hp}")
                e0 = den0[0:1, :]
                e1 = avsO[0:1, :]
                nc.sync.dma_start(
                    out=denb[0:D, :],
                    in_=bass.AP(tensor=e0.tensor, offset=e0.offset,
                                ap=[[1, 1], [0, D], [1, TCH]]))
                nc.sync.dma_start(
                    out=denb[D:P, :],
                    in_=bass.AP(tensor=e1.tensor, offset=e1.offset,
                                ap=[[1, 1], [0, D], [1, TCH]]))
                rdb = rdbp.tile([P, TCH], F32, tag="rdb",
                                name=f"rdb_{tci}_{hp}")
                nc.vector.reciprocal_approx_fast(out=rdb, in_=denb)
                nc.vector.tensor_mul(aTn[hp][0:D, :], avsE[0:D, :],
                                     rdb[0:D, :])
                nc.vector.tensor_mul(aTn[hp][D:P, :], avsO[D:P, :],
                                     rdb[D:P, :])

            def emit_outproj():
                for ts in range(4):
                    pf = pmix.tile([P, E], F32, tag="pf",
                                   name=f"pf_{tci}_{ts}")
                    for hp2 in range(2):
                        nc.tensor.matmul(pf,
                                         aTn[hp2][:, ts * P:(ts + 1) * P],
                                         wo_sb[hp2],
                                         start=(hp2 == 0), stop=(hp2 == 1))
                    ot = outp.tile([P, E], F32, tag="ot",
                                   name=f"ot_{tci}_{ts}")
                    nc.vector.tensor_copy(ot, pf)
                    row = (tci * 4 + ts) * P
                    nc.sync.dma_start(out=out[row:row + P, :], in_=ot)

            return emit_outproj

        with ExitStack() as cp:
            psp = cp.enter_context(
                tc.tile_pool(name="psp", bufs=2, space="PSUM"))
            sig = cp.enter_context(tc.tile_pool(name="sig", bufs=4))
            for tcq in range(NTC):
                proj_group(tcq, psp, sig)
        with ExitStack() as c4:
            pss = c4.enter_context(
                tc.tile_pool(name="pss", bufs=2, space="PSUM"))
            pmix = c4.enter_context(
                tc.tile_pool(name="pmix", bufs=1, space="PSUM"))
            pending = None
            for tcq in range(NTC):
                pending = attention_tci(tcq, pss, pmix, pending)
            pending()

    nc.compile()
    return nc


_NC = None
_LAST_IN_MAPS = None


def _get_nc():
    global _NC
    if _NC is None:
        _NC = _build()
    return _NC


def kernel(**inputs):
    query = np.asarray(inputs["query"], np.float32)
    key_ = np.asarray(inputs["key_"] if "key_" in inputs else inputs["key"],
                      np.float32)
    value = np.asarray(inputs["value"], np.float32)
    Wq = np.asarray(inputs["Wq"], np.float32)
    bq = np.asarray(inputs["bq"], np.float32)
    Wk = np.asarray(inputs["Wk"], np.float32)
    bk = np.asarray(inputs["bk"], np.float32)
    Wv = np.asarray(inputs["Wv"], np.float32)
    bv = np.asarray(inputs["bv"], np.float32)
    Wo = np.asarray(inputs["Wo"], np.float32)
    bo = np.asarray(inputs["bo"], np.float32)
    r = float(np.asarray(inputs["r"]).reshape(-1)[0])

    r_s = 4.0 / (1.0 + np.exp(-np.float64(r)))
    c = np.float32(r_s * r_s / 8.0)

    WqT = Wq.T.astype(BF16NP)
    WkT = Wk.T.astype(BF16NP)
    WoT = Wo.T.astype(BF16NP)
    WvTa = np.concatenate([Wv.T, bv[None, :]], axis=0).astype(BF16NP)

    in_maps = []
    for b in range(B):
        xqT = np.ascontiguousarray(query[b].T).astype(BF16NP)
        xkT = np.ascontiguousarray(key_[b].T).astype(BF16NP)
        xvT = np.concatenate(
            [np.ascontiguousarray(value[b].T), np.ones((1, S), np.float32)],
            axis=0).astype(BF16NP)
        for g in range(HG):
            cols = slice(g * EG, (g + 1) * EG)
            in_maps.append(dict(
                xqT=xqT, xkT=xkT, xvT=xvT,
                wqT=np.ascontiguousarray(WqT[:, cols]),
                wkT=np.ascontiguousarray(WkT[:, cols]),
                wvT=np.ascontiguousarray(WvTa[:, cols]),
                woT=np.ascontiguousarray(WoT[cols, :]),
                bq=np.ascontiguousarray(0.5 * bq[cols, None]),
                bk=np.ascontiguousarray(0.5 * bk[cols, None]),
                cexp=np.array([[c]], np.float32),
            ))

    global _LAST_IN_MAPS
    _LAST_IN_MAPS = in_maps
    res = run_bass_kernel_spmd(_get_nc(), in_maps, core_ids=list(range(NCORES)))
    out = np.empty((B, T, E), np.float32)
    for b in range(B):
        out[b] = res.results[HG * b]["out"]
        for g in range(1, HG):
            out[b] += res.results[HG * b + g]["out"]
        out[b] += bo[None, :]
    return out


# revision 9
# speedup vs baseline: 1.1228x; 1.0130x over previous
"""Trainium2 Bass kernel for ChaoticAttentionLayer.

Math (reference):
    q = r_s * sig(zq) * (1 - sig(zq)),  zq = query @ Wq.T + bq,  r_s = 4*sigmoid(r)
    k likewise, v = value @ Wv.T + bv
    out = softmax(q k^T / 8) v @ Wo.T + bo   (per head, D=64)

Device decomposition:
    g = sig*(1-sig); scores = (r_s^2/8) * g(zq) . g(zk); the r_s^2/8 factor is
    folded into the Exp activation's scale. Scores are bounded in [0, 8] for
    any r, so softmax runs max-free: exp(scores) directly.

Sharding: 8 cores = 4 batches x 2 head-groups (4 heads each). Each core
computes partial out[b] = attn_hg @ Wo[:, hg].T; host sums the two partials
per batch and adds bo.

Structure (per core, all bf16 on matmul paths):
  - Scores are computed transposed, S^T[s_block, t], two heads per Exp call
    (row-tiled 64-contraction matmuls run concurrently on the PE).
  - attn@V uses V as the stationary operand: av[d, t] = V'^T ex, one
    N=512 matmul per (head, s_block). V' is padded per head to 128 cols:
    even head [V | 1 | 0*63], odd head [0*63 | 1 | V], so the even head's
    numerator lands at PSUM partitions 0..63 and the odd head's at 64..127.
    The softmax denominator comes out at row 64 / 63 respectively.
  - Normalization: 1/den via reciprocal_approx_fast on the den row, DMA
    partition-broadcast of the reciprocal row, then one tensor_mul per head
    writes the normalized, already-transposed attention output aTn[128, t]
    (both heads of a pair stacked) -- no PE transposes needed.
  - Out-projection: full K=128 contraction lhsT=aTn, accumulated over the
    two head pairs; final scale+copy and DMA out.
"""

import numpy as np
import ml_dtypes
from contextlib import ExitStack

try:
    import concourse.bass as bass
except ImportError:  # pragma: no cover
    import sys

    sys.path.insert(0, "/opt/trn_rl_repo")
    import concourse.bass as bass

import concourse.bacc as bacc
import concourse.tile as tile
from concourse import mybir
from concourse.bass_utils import run_bass_kernel_spmd

F32 = mybir.dt.float32
BF16 = mybir.dt.bfloat16
AF = mybir.ActivationFunctionType
BF16NP = ml_dtypes.bfloat16

B, T, S, E, H = 4, 2048, 2048, 512, 8
D = E // H           # 64 head dim
HG = 2               # head-groups per batch (cores per batch)
HPG = H // HG        # 4 heads per group
EG = HPG * D         # 256 dims per head group
NCORES = 8
P = 128              # partitions
TCH = 512            # t-chunk (psum free dim)
NSB = S // P         # 16 s-blocks
NKT = E // P         # 4 contraction tiles of 128
NTC = T // TCH       # 4 t-chunks


def _build():
    nc = bacc.Bacc("TRN2", target_bir_lowering=False, debug=False,
                   num_devices=NCORES)

    xqT = nc.dram_tensor("xqT", [E, T], BF16, kind="ExternalInput")
    xkT = nc.dram_tensor("xkT", [E, S], BF16, kind="ExternalInput")
    xvT = nc.dram_tensor("xvT", [E + 1, S], BF16, kind="ExternalInput")
    wqT = nc.dram_tensor("wqT", [E, EG], BF16, kind="ExternalInput")
    wkT = nc.dram_tensor("wkT", [E, EG], BF16, kind="ExternalInput")
    wvT = nc.dram_tensor("wvT", [E + 1, EG], BF16, kind="ExternalInput")
    woT = nc.dram_tensor("woT", [EG, E], BF16, kind="ExternalInput")
    bq = nc.dram_tensor("bq", [EG, 1], F32, kind="ExternalInput")
    bk = nc.dram_tensor("bk", [EG, 1], F32, kind="ExternalInput")
    cexp = nc.dram_tensor("cexp", [1, 1], F32, kind="ExternalInput")
    out = nc.dram_tensor("out", [T, E], F32, kind="ExternalOutput")

    with tile.TileContext(nc) as tc, ExitStack() as ctx:
        persist = ctx.enter_context(tc.tile_pool(name="persist", bufs=1))

        # --- persistent SBUF state ---
        # K-projection inputs first: they gate the whole pipeline.
        wk_sb = []
        bk_sb = []
        for kt in range(NKT):
            tk = persist.tile([P, EG], BF16, tag=f"wk{kt}")
            nc.sync.dma_start(out=tk, in_=wkT[kt * P:(kt + 1) * P, :])
            wk_sb.append(tk)
        for c in range(EG // P):
            tb2 = persist.tile([P, 1], F32, tag=f"bk{c}")
            nc.sync.dma_start(out=tb2, in_=bk[c * P:(c + 1) * P, :])
            bk_sb.append(tb2)
        # projected tensors, resident for the whole kernel; chunked into
        # [P, TCH] column tiles so consumers unblock per-chunk.
        QT_sb = [[persist.tile([P, TCH], BF16, tag=f"qt{c}_{q}",
                               name=f"qt{c}_{q}") for q in range(NTC)]
                 for c in range(EG // P)]
        KT_sb = [[persist.tile([P, TCH], BF16, tag=f"kt{c}_{q}",
                               name=f"ktile{c}_{q}") for q in range(NTC)]
                 for c in range(EG // P)]
        # V', padded to 128 cols per head:
        #   even head h: [V_h (64) | ones (1) | zeros (63)]
        #   odd  head h: [ones (1) | zeros (63) | V_h (64)]
        V2_sb = [persist.tile([P, HPG, P], BF16, tag=f"v{sc}", name=f"v{sc}")
                 for sc in range(NSB)]

        # --- x inputs + remaining weights, in consumption order ---
        xk_sb = [[None] * NTC for _ in range(NKT)]
        xv_sb = [[None] * NTC for _ in range(NKT)]
        xq_sb = [[None] * NTC for _ in range(NKT)]

        def load_x(dst, src, kt, q):
            t = persist.tile([P, TCH], BF16, tag=f"{dst}{kt}_{q}",
                             name=f"{dst}{kt}_{q}")
            nc.sync.dma_start(
                out=t, in_=src[kt * P:(kt + 1) * P, q * TCH:(q + 1) * TCH])
            return t

        for kt in range(NKT):
            xk_sb[kt][0] = load_x("xk", xkT, kt, 0)

        wq_sb = []
        bq_sb = []
        for kt in range(NKT):
            tq = persist.tile([P, EG], BF16, tag=f"wq{kt}")
            nc.sync.dma_start(out=tq, in_=wqT[kt * P:(kt + 1) * P, :])
            wq_sb.append(tq)
        for c in range(EG // P):
            tb_ = persist.tile([P, 1], F32, tag=f"bq{c}")
            nc.sync.dma_start(out=tb_, in_=bq[c * P:(c + 1) * P, :])
            bq_sb.append(tb_)
        for kt in range(NKT):
            xq_sb[kt][0] = load_x("xq", xqT, kt, 0)

        wv_sb = []
        for kt in range(NKT):
            tv = persist.tile([P, EG], BF16, tag=f"wv{kt}")
            nc.sync.dma_start(out=tv, in_=wvT[kt * P:(kt + 1) * P, :])
            wv_sb.append(tv)
        wv4_sb = persist.tile([1, EG], BF16, tag="wv4")
        nc.sync.dma_start(out=wv4_sb, in_=wvT[E:E + 1, :])
        ones_sb = persist.tile([1, S], BF16, tag="ones")
        nc.sync.dma_start(out=ones_sb, in_=xvT[E:E + 1, :])
        for kt in range(NKT):
            xv_sb[kt][0] = load_x("xvr", xvT, kt, 0)

        cexp_sb = persist.tile([P, 1], F32, tag="cexp")
        cap = cexp[:, :]
        nc.sync.dma_start(
            out=cexp_sb,
            in_=bass.AP(tensor=cap.tensor, offset=cap.offset, ap=[[0, P], [1, 1]]),
        )

        for q in range(1, NTC):
            for kt in range(NKT):
                xk_sb[kt][q] = load_x("xk", xkT, kt, q)
            for kt in range(NKT):
                xq_sb[kt][q] = load_x("xq", xqT, kt, q)
            for kt in range(NKT):
                xv_sb[kt][q] = load_x("xvr", xvT, kt, q)

        # out-proj weights are only needed much later.
        wo_sb = []
        for kb in range(EG // P):
            to = persist.tile([P, E], BF16, tag=f"wo{kb}")
            nc.sync.dma_start(out=to, in_=woT[kb * P:(kb + 1) * P, :])
            wo_sb.append(to)

        def qk_proj_chunk(pool, x_sb, w_sb, b_sb, out_tiles, c, tcq, sig):
            ps = pool.tile([P, TCH], F32, tag="ps", name=f"ps_{c}_{tcq}")
            for kt in range(NKT):
                nc.tensor.matmul(
                    ps, w_sb[kt][:, c * P:(c + 1) * P],
                    x_sb[kt][tcq],
                    start=(kt == 0), stop=(kt == NKT - 1))
            # sig'(z) = (1 - tanh^2(z/2)) / 4 -- tanh shares ACT's exp
            # table set, so the whole kernel needs one ACT_TABLE_LOAD.
            y = sig.tile([P, TCH], F32, tag="y", name=f"y_{c}_{tcq}")
            nc.scalar.activation(y, ps, AF.Tanh, bias=b_sb[c], scale=0.5)
            y2 = sig.tile([P, TCH], F32, tag="y2", name=f"y2_{c}_{tcq}")
            nc.vector.tensor_mul(y2, y, y)
            nc.vector.tensor_scalar(out_tiles[c][tcq], y2, -0.25, 0.25,
                                    mybir.AluOpType.mult,
                                    mybir.AluOpType.add)

        def proj_group(tcq, psp, sig):
            for c in range(EG // P):
                qk_proj_chunk(psp, xk_sb, wk_sb, bk_sb, KT_sb, c, tcq, sig)
            for c in range(EG // P):
                qk_proj_chunk(psp, xq_sb, wq_sb, bq_sb, QT_sb, c, tcq, sig)
            for sci in range(4):
                sc = tcq * 4 + sci
                ps = psp.tile([P, TCH], F32, tag="ps", name=f"psv_{sc}")
                for kt in range(NKT):
                    nc.tensor.matmul(ps[:, 0:EG],
                                     xv_sb[kt][tcq][:, sci * P:(sci + 1) * P],
                                     wv_sb[kt], start=(kt == 0), stop=False)
                nc.tensor.matmul(ps[:, 0:EG], ones_sb[:, sc * P:(sc + 1) * P],
                                 wv4_sb, start=False, stop=True)
                v2 = V2_sb[sc]
                psv = ps[:, 0:EG].rearrange("p (h d) -> p h d", h=HPG)
                # even heads: V at cols 0..63, ones at 64, zeros above
                nc.vector.tensor_copy(v2[:, 0::2, 0:D], psv[:, 0::2, :])
                nc.vector.memset(v2[:, 0::2, D:D + 1], 1.0)
                nc.vector.memset(v2[:, 0::2, D + 1:P], 0.0)
                # odd heads: ones at 0, zeros at 1..63, V at cols 64..127
                nc.vector.tensor_copy(v2[:, 1::2, D:P], psv[:, 1::2, :])
                nc.vector.memset(v2[:, 1::2, 0:1], 1.0)
                nc.vector.memset(v2[:, 1::2, 1:D], 0.0)

        # --- attention + out-projection ---
        expp = ctx.enter_context(tc.tile_pool(name="expp", bufs=3))
        aTnp = ctx.enter_context(tc.tile_pool(name="aTnp", bufs=2))
        denp = ctx.enter_context(tc.tile_pool(name="denp", bufs=2))
        rdbp = ctx.enter_context(tc.tile_pool(name="rdbp", bufs=2))
        outp = ctx.enter_context(tc.tile_pool(name="outp", bufs=3))

        def attention_tci(tci, pss, pmix, pending):
            aTn = [aTnp.tile([P, TCH], BF16, tag=f"aTn{hp}",
                             name=f"aTn_{tci}_{hp}") for hp in range(2)]
            for hp in range(2):
                ph = tci * 2 + hp
                avE = pmix.tile([P, TCH], F32, tag=f"av{(2 * ph) % 3}",
                                name=f"avE_{tci}_{hp}")
                avO = pmix.tile([P, TCH], F32, tag=f"av{(2 * ph + 1) % 3}",
                                name=f"avO_{tci}_{hp}")
                avs = [avE, avO]
                exs = [None] * NSB

                def emit_av(sb):
                    for hi in range(2):
                        h = 2 * hp + hi
                        nc.tensor.matmul(
                            avs[hi],
                            V2_sb[sb][:, h, :],
                            exs[sb][:, hi * TCH:(hi + 1) * TCH],
                            start=(sb == 0), stop=(sb == NSB - 1),
                            skip_group_check=(sb != 0))

                for sb in range(NSB):
                    ps = pss.tile([P, 2 * TCH], F32, tag="ps",
                                  name=f"ps_{tci}_{hp}_{sb}")
                    for hi in range(2):
                        h = 2 * hp + hi
                        off = hi * D
                        nc.tensor.matmul(
                            ps[:, hi * TCH:(hi + 1) * TCH],
                            KT_sb[hp][sb // 4][off:off + D,
                                               (sb % 4) * P:(sb % 4 + 1) * P],
                            QT_sb[hp][tci][off:off + D, :],
                            start=True, stop=True,
                            tile_position=(off, 0))
                    ex = expp.tile([P, 2 * TCH], BF16, tag="ex",
                                   name=f"ex_{tci}_{hp}_{sb}")
                    nc.scalar.activation(ex, ps, AF.Exp, scale=cexp_sb)
                    exs[sb] = ex
                    if sb == 12 and hp == 0 and pending is not None:
                        pending()
                    if sb >= 1:
                        emit_av(sb - 1)
                emit_av(NSB - 1)

                # Copy both av banks to SBUF immediately so the PSUM banks
                # free up for the next phase; the (DMA-latency-heavy) norm
                # chain then runs entirely off the PE critical path.
                avsE = denp.tile([P, TCH], F32, tag="avsE",
                                 name=f"avsE_{tci}_{hp}")
                avsO = denp.tile([P, TCH], F32, tag="avsO",
                                 name=f"avsO_{tci}_{hp}")
                nc.vector.tensor_copy(avsE[0:D + 1, :], avE[0:D + 1, :])
                nc.vector.tensor_copy(avsO, avO)
                # normalization: shift the even-head den row to partition 0
                # via DMA, broadcast both den rows, one reciprocal, scale.
                den0 = denp.tile([1, TCH], F32, tag="den0",
                                 name=f"den0_{tci}_{hp}")
                nc.sync.dma_start(out=den0, in_=avsE[D:D + 1, :])
                denb = rdbp.tile([P, TCH], F32, tag="denb",
                                 name=f"denb_{tci}_{hp}")
                e0 = den0[0:1, :]
                e1 = avsO[0:1, :]
                nc.sync.dma_start(
                    out=denb[0:D, :],
                    in_=bass.AP(tensor=e0.tensor, offset=e0.offset,
                                ap=[[1, 1], [0, D], [1, TCH]]))
                nc.sync.dma_start(
                    out=denb[D:P, :],
                    in_=bass.AP(tensor=e1.tensor, offset=e1.offset,
                                ap=[[1, 1], [0, D], [1, TCH]]))
                rdb = rdbp.tile([P, TCH], F32, tag="rdb",
                                name=f"rdb_{tci}_{hp}")
                nc.vector.reciprocal_approx_fast(out=rdb, in_=denb)
                nc.vector.tensor_mul(aTn[hp][0:D, :], avsE[0:D, :],
                                     rdb[0:D, :])
                nc.vector.tensor_mul(aTn[hp][D:P, :], avsO[D:P, :],
                                     rdb[D:P, :])

            def emit_outproj():
                for ts in range(4):
                    pf = pmix.tile([P, E], F32, tag="pf",
                                   name=f"pf_{tci}_{ts}")
                    for hp2 in range(2):
                        nc.tensor.matmul(pf,
                                         aTn[hp2][:, ts * P:(ts + 1) * P],
                                         wo_sb[hp2],
                                         start=(hp2 == 0), stop=(hp2 == 1))
                    ot = outp.tile([P, E], F32, tag="ot",
                                   name=f"ot_{tci}_{ts}")
                    nc.vector.tensor_copy(ot, pf)
                    row = (tci * 4 + ts) * P
                    nc.sync.dma_start(out=out[row:row + P, :], in_=ot)

            return emit_outproj

        with ExitStack() as cp:
            psp = cp.enter_context(
                tc.tile_pool(name="psp", bufs=2, space="PSUM"))
            sig = cp.enter_context(tc.tile_pool(name="sig", bufs=4))
            for tcq in range(NTC):
                proj_group(tcq, psp, sig)
        with ExitStack() as c4:
            pss = c4.enter_context(
                tc.tile_pool(name="pss", bufs=2, space="PSUM"))
            pmix = c4.enter_context(
                tc.tile_pool(name="pmix", bufs=1, space="PSUM"))
            pending = None
            for tcq in range(NTC):
                pending = attention_tci(tcq, pss, pmix, pending)
            pending()

    nc.compile()
    return nc


_NC = None
_LAST_IN_MAPS = None


def _get_nc():
    global _NC
    if _NC is None:
        _NC = _build()
    return _NC


def kernel(**inputs):
    query = np.asarray(inputs["query"], np.float32)
    key_ = np.asarray(inputs["key_"] if "key_" in inputs else inputs["key"],
                      np.float32)
    value = np.asarray(inputs["value"], np.float32)
    Wq = np.asarray(inputs["Wq"], np.float32)
    bq = np.asarray(inputs["bq"], np.float32)
    Wk = np.asarray(inputs["Wk"], np.float32)
    bk = np.asarray(inputs["bk"], np.float32)
    Wv = np.asarray(inputs["Wv"], np.float32)
    bv = np.asarray(inputs["bv"], np.float32)
    Wo = np.asarray(inputs["Wo"], np.float32)
    bo = np.asarray(inputs["bo"], np.float32)
    r = float(np.asarray(inputs["r"]).reshape(-1)[0])

    r_s = 4.0 / (1.0 + np.exp(-np.float64(r)))
    c = np.float32(r_s * r_s / 8.0)

    WqT = Wq.T.astype(BF16NP)
    WkT = Wk.T.astype(BF16NP)
    WoT = Wo.T.astype(BF16NP)
    WvTa = np.concatenate([Wv.T, bv[None, :]], axis=0).astype(BF16NP)

    in_maps = []
    for b in range(B):
        xqT = np.ascontiguousarray(query[b].T).astype(BF16NP)
        xkT = np.ascontiguousarray(key_[b].T).astype(BF16NP)
        xvT = np.concatenate(
            [np.ascontiguousarray(value[b].T), np.ones((1, S), np.float32)],
            axis=0).astype(BF16NP)
        for g in range(HG):
            cols = slice(g * EG, (g + 1) * EG)
            in_maps.append(dict(
                xqT=xqT, xkT=xkT, xvT=xvT,
                wqT=np.ascontiguousarray(WqT[:, cols]),
                wkT=np.ascontiguousarray(WkT[:, cols]),
                wvT=np.ascontiguousarray(WvTa[:, cols]),
                woT=np.ascontiguousarray(WoT[cols, :]),
                bq=np.ascontiguousarray(0.5 * bq[cols, None]),
                bk=np.ascontiguousarray(0.5 * bk[cols, None]),
                cexp=np.array([[c]], np.float32),
            ))

    global _LAST_IN_MAPS
    _LAST_IN_MAPS = in_maps
    res = run_bass_kernel_spmd(_get_nc(), in_maps, core_ids=list(range(NCORES)))
    out = np.empty((B, T, E), np.float32)
    for b in range(B):
        out[b] = res.results[HG * b]["out"]
        for g in range(1, HG):
            out[b] += res.results[HG * b + g]["out"]
        out[b] += bo[None, :]
    return out


# revision 10
# speedup vs baseline: 1.4268x; 1.2707x over previous
"""Trainium2 Bass kernel for ChaoticAttentionLayer.

Math (reference):
    q = r_s * sig(zq) * (1 - sig(zq)),  zq = query @ Wq.T + bq,  r_s = 4*sigmoid(r)
    k likewise, v = value @ Wv.T + bv
    out = softmax(q k^T / 8) v @ Wo.T + bo   (per head, D=64)

Device decomposition:
    g = sig*(1-sig); scores = (r_s^2/8) * g(zq) . g(zk); the r_s^2/8 factor is
    folded into the Exp activation's scale. Scores are bounded in [0, 8] for
    any r, so softmax runs max-free: exp(scores) directly.

Sharding: 8 cores = 4 batches x 2 head-groups (4 heads each). Each core
computes partial out[b] = attn_hg @ Wo[:, hg].T; host sums the two partials
per batch and adds bo.

Structure (per core, all bf16 on matmul paths):
  - Scores are computed transposed, S^T[s_block, t], two heads per Exp call
    (row-tiled 64-contraction matmuls run concurrently on the PE).
  - attn@V uses V as the stationary operand: av[d, t] = V'^T ex, one
    N=512 matmul per (head, s_block). V' is padded per head to 128 cols:
    even head [V | 1 | 0*63], odd head [0*63 | 1 | V], so the even head's
    numerator lands at PSUM partitions 0..63 and the odd head's at 64..127.
    The softmax denominator comes out at row 64 / 63 respectively.
  - Normalization: 1/den via reciprocal_approx_fast on the den row, DMA
    partition-broadcast of the reciprocal row, then one tensor_mul per head
    writes the normalized, already-transposed attention output aTn[128, t]
    (both heads of a pair stacked) -- no PE transposes needed.
  - Out-projection: full K=128 contraction lhsT=aTn, accumulated over the
    two head pairs; final scale+copy and DMA out.
"""

import numpy as np
import ml_dtypes
from contextlib import ExitStack

try:
    import concourse.bass as bass
except ImportError:  # pragma: no cover
    import sys

    sys.path.insert(0, "/opt/trn_rl_repo")
    import concourse.bass as bass

import concourse.bacc as bacc
import concourse.tile as tile
from concourse import mybir
from concourse.bass_utils import run_bass_kernel_spmd

F32 = mybir.dt.float32
BF16 = mybir.dt.bfloat16
AF = mybir.ActivationFunctionType
BF16NP = ml_dtypes.bfloat16

B, T, S, E, H = 4, 2048, 2048, 512, 8
D = E // H           # 64 head dim
HG = 2               # head-groups per batch (cores per batch)
HPG = H // HG        # 4 heads per group
EG = HPG * D         # 256 dims per head group
NCORES = 8
P = 128              # partitions
TCH = 512            # t-chunk (psum free dim)
NSB = S // P         # 16 s-blocks
NKT = E // P         # 4 contraction tiles of 128
NTC = T // TCH       # 4 t-chunks


def _build():
    nc = bacc.Bacc("TRN2", target_bir_lowering=False, debug=False,
                   num_devices=NCORES)

    xqT = nc.dram_tensor("xqT", [E, T], BF16, kind="ExternalInput")
    xkT = nc.dram_tensor("xkT", [E, S], BF16, kind="ExternalInput")
    xvT = nc.dram_tensor("xvT", [E + 1, S], BF16, kind="ExternalInput")
    wqT = nc.dram_tensor("wqT", [E, EG], BF16, kind="ExternalInput")
    wkT = nc.dram_tensor("wkT", [E, EG], BF16, kind="ExternalInput")
    wvT = nc.dram_tensor("wvT", [E + 1, EG], BF16, kind="ExternalInput")
    woT = nc.dram_tensor("woT", [EG, E], BF16, kind="ExternalInput")
    bq = nc.dram_tensor("bq", [EG, 1], F32, kind="ExternalInput")
    bk = nc.dram_tensor("bk", [EG, 1], F32, kind="ExternalInput")
    cexp = nc.dram_tensor("cexp", [1, 1], F32, kind="ExternalInput")
    out = nc.dram_tensor("out", [T, E], F32, kind="ExternalOutput")

    with tile.TileContext(nc) as tc, ExitStack() as ctx:
        persist = ctx.enter_context(tc.tile_pool(name="persist", bufs=1))

        # --- persistent SBUF state ---
        # K-projection inputs first: they gate the whole pipeline.
        wk_sb = []
        bk_sb = []
        for kt in range(NKT):
            tk = persist.tile([P, EG], BF16, tag=f"wk{kt}")
            nc.sync.dma_start(out=tk, in_=wkT[kt * P:(kt + 1) * P, :])
            wk_sb.append(tk)
        for c in range(EG // P):
            tb2 = persist.tile([P, 1], F32, tag=f"bk{c}")
            nc.sync.dma_start(out=tb2, in_=bk[c * P:(c + 1) * P, :])
            bk_sb.append(tb2)
        # projected tensors, resident for the whole kernel; chunked into
        # [P, TCH] column tiles so consumers unblock per-chunk.
        QT_sb = [[persist.tile([P, TCH], BF16, tag=f"qt{c}_{q}",
                               name=f"qt{c}_{q}") for q in range(NTC)]
                 for c in range(EG // P)]
        KT_sb = [[persist.tile([P, TCH], BF16, tag=f"kt{c}_{q}",
                               name=f"ktile{c}_{q}") for q in range(NTC)]
                 for c in range(EG // P)]
        # V', padded to 128 cols per head:
        #   even head h: [V_h (64) | ones (1) | zeros (63)]
        #   odd  head h: [ones (1) | zeros (63) | V_h (64)]
        V2_sb = [persist.tile([P, HPG, P], BF16, tag=f"v{sc}", name=f"v{sc}")
                 for sc in range(NSB)]

        # --- x inputs + remaining weights, in consumption order ---
        xk_sb = [[None] * NTC for _ in range(NKT)]
        xv_sb = [[None] * NTC for _ in range(NKT)]
        xq_sb = [[None] * NTC for _ in range(NKT)]

        def load_x(dst, src, kt, q):
            t = persist.tile([P, TCH], BF16, tag=f"{dst}{kt}_{q}",
                             name=f"{dst}{kt}_{q}")
            nc.sync.dma_start(
                out=t, in_=src[kt * P:(kt + 1) * P, q * TCH:(q + 1) * TCH])
            return t

        for kt in range(NKT):
            xk_sb[kt][0] = load_x("xk", xkT, kt, 0)

        wq_sb = []
        bq_sb = []
        for kt in range(NKT):
            tq = persist.tile([P, EG], BF16, tag=f"wq{kt}")
            nc.sync.dma_start(out=tq, in_=wqT[kt * P:(kt + 1) * P, :])
            wq_sb.append(tq)
        for c in range(EG // P):
            tb_ = persist.tile([P, 1], F32, tag=f"bq{c}")
            nc.sync.dma_start(out=tb_, in_=bq[c * P:(c + 1) * P, :])
            bq_sb.append(tb_)
        for kt in range(NKT):
            xq_sb[kt][0] = load_x("xq", xqT, kt, 0)

        wv_sb = []
        for kt in range(NKT):
            tv = persist.tile([P, EG], BF16, tag=f"wv{kt}")
            nc.sync.dma_start(out=tv, in_=wvT[kt * P:(kt + 1) * P, :])
            wv_sb.append(tv)
        wv4_sb = persist.tile([1, EG], BF16, tag="wv4")
        nc.sync.dma_start(out=wv4_sb, in_=wvT[E:E + 1, :])
        ones_sb = persist.tile([1, S], BF16, tag="ones")
        nc.sync.dma_start(out=ones_sb, in_=xvT[E:E + 1, :])
        for kt in range(NKT):
            xv_sb[kt][0] = load_x("xvr", xvT, kt, 0)

        ones64 = persist.tile([P, D], BF16, tag="ones64")
        nc.vector.memset(ones64, 1.0)

        cexp_sb = persist.tile([P, 1], F32, tag="cexp")
        cap = cexp[:, :]
        nc.sync.dma_start(
            out=cexp_sb,
            in_=bass.AP(tensor=cap.tensor, offset=cap.offset, ap=[[0, P], [1, 1]]),
        )

        for q in range(1, NTC):
            for kt in range(NKT):
                xk_sb[kt][q] = load_x("xk", xkT, kt, q)
            for kt in range(NKT):
                xq_sb[kt][q] = load_x("xq", xqT, kt, q)
            for kt in range(NKT):
                xv_sb[kt][q] = load_x("xvr", xvT, kt, q)

        # out-proj weights are only needed much later.
        wo_sb = []
        for kb in range(EG // P):
            to = persist.tile([P, E], BF16, tag=f"wo{kb}")
            nc.sync.dma_start(out=to, in_=woT[kb * P:(kb + 1) * P, :])
            wo_sb.append(to)

        def qk_proj_chunk(pool, x_sb, w_sb, b_sb, out_tiles, c, tcq, sig):
            ps = pool.tile([P, TCH], F32, tag="ps", name=f"ps_{c}_{tcq}")
            for kt in range(NKT):
                nc.tensor.matmul(
                    ps, w_sb[kt][:, c * P:(c + 1) * P],
                    x_sb[kt][tcq],
                    start=(kt == 0), stop=(kt == NKT - 1))
            # sig'(z) = (1 - tanh^2(z/2)) / 4 -- tanh shares ACT's exp
            # table set, so the whole kernel needs one ACT_TABLE_LOAD.
            y = sig.tile([P, TCH], F32, tag="y", name=f"y_{c}_{tcq}")
            nc.scalar.activation(y, ps, AF.Tanh, bias=b_sb[c], scale=0.5)
            y2 = sig.tile([P, TCH], F32, tag="y2", name=f"y2_{c}_{tcq}")
            nc.vector.tensor_mul(y2, y, y)
            nc.vector.tensor_scalar(out_tiles[c][tcq], y2, -0.25, 0.25,
                                    mybir.AluOpType.mult,
                                    mybir.AluOpType.add)

        def proj_group(tcq, psp, sig):
            for c in range(EG // P):
                qk_proj_chunk(psp, xk_sb, wk_sb, bk_sb, KT_sb, c, tcq, sig)
            for c in range(EG // P):
                qk_proj_chunk(psp, xq_sb, wq_sb, bq_sb, QT_sb, c, tcq, sig)
            for sci in range(4):
                sc = tcq * 4 + sci
                ps = psp.tile([P, TCH], F32, tag="ps", name=f"psv_{sc}")
                for kt in range(NKT):
                    nc.tensor.matmul(ps[:, 0:EG],
                                     xv_sb[kt][tcq][:, sci * P:(sci + 1) * P],
                                     wv_sb[kt], start=(kt == 0), stop=False)
                nc.tensor.matmul(ps[:, 0:EG], ones_sb[:, sc * P:(sc + 1) * P],
                                 wv4_sb, start=False, stop=True)
                v2 = V2_sb[sc]
                psv = ps[:, 0:EG].rearrange("p (h d) -> p h d", h=HPG)
                # even heads: V at cols 0..63, ones at 64, zeros above
                nc.vector.tensor_copy(v2[:, 0::2, 0:D], psv[:, 0::2, :])
                nc.vector.memset(v2[:, 0::2, D:D + 1], 1.0)
                nc.vector.memset(v2[:, 0::2, D + 1:P], 0.0)
                # odd heads: ones at 0, zeros at 1..63, V at cols 64..127
                nc.vector.tensor_copy(v2[:, 1::2, D:P], psv[:, 1::2, :])
                nc.vector.memset(v2[:, 1::2, 0:1], 1.0)
                nc.vector.memset(v2[:, 1::2, 1:D], 0.0)

        # --- attention + out-projection ---
        expp = ctx.enter_context(tc.tile_pool(name="expp", bufs=3))
        aTnp = ctx.enter_context(tc.tile_pool(name="aTnp", bufs=2))
        avsp = ctx.enter_context(tc.tile_pool(name="avsp", bufs=2))
        rdbp = ctx.enter_context(tc.tile_pool(name="rdbp", bufs=2))
        outp = ctx.enter_context(tc.tile_pool(name="outp", bufs=3))

        # deferred-emission state: norm-chain PE ops and out-projection are
        # emitted a few s-blocks into the NEXT phase so the in-order PE queue
        # never stalls on them.
        state = {"norm": None, "out": None}

        def attention_tci(tci, pss, pmix):
            aTn = [aTnp.tile([P, TCH], BF16, tag=f"aTn{hp}",
                             name=f"aTn_{tci}_{hp}") for hp in range(2)]
            for hp in range(2):
                avE = pmix.tile([P, TCH], F32, tag="av0",
                                name=f"avE_{tci}_{hp}")
                avO = pmix.tile([P, TCH], F32, tag="av1",
                                name=f"avO_{tci}_{hp}")
                avs = [avE, avO]
                exs = [None] * NSB

                def emit_av(sb, avs=avs, exs=exs, hp=hp):
                    for hi in range(2):
                        h = 2 * hp + hi
                        nc.tensor.matmul(
                            avs[hi],
                            V2_sb[sb][:, h, :],
                            exs[sb][:, hi * TCH:(hi + 1) * TCH],
                            start=(sb == 0), stop=(sb == NSB - 1),
                            skip_group_check=(sb != 0))

                for sb in range(NSB):
                    ps = pss.tile([P, 2 * TCH], F32, tag="ps",
                                  name=f"ps_{tci}_{hp}_{sb}")
                    for hi in range(2):
                        h = 2 * hp + hi
                        off = hi * D
                        nc.tensor.matmul(
                            ps[:, hi * TCH:(hi + 1) * TCH],
                            KT_sb[hp][sb // 4][off:off + D,
                                               (sb % 4) * P:(sb % 4 + 1) * P],
                            QT_sb[hp][tci][off:off + D, :],
                            start=True, stop=True,
                            tile_position=(off, 0))
                    ex = expp.tile([P, 2 * TCH], BF16, tag="ex",
                                   name=f"ex_{tci}_{hp}_{sb}")
                    nc.scalar.activation(ex, ps, AF.Exp, scale=cexp_sb)
                    exs[sb] = ex
                    if sb == 2 and state["norm"] is not None:
                        state["norm"]()
                        state["norm"] = None
                    if sb == 12 and hp == 0 and state["out"] is not None:
                        state["out"]()
                        state["out"] = None
                    if sb >= 1:
                        emit_av(sb - 1)
                emit_av(NSB - 1)

                # copy both av banks to SBUF (bf16) immediately: frees the
                # PSUM banks for the next phase and provides SBUF operands
                # for the PE den-broadcast matmuls.
                avsE = avsp.tile([P, TCH], BF16, tag="avsE",
                                 name=f"avsE_{tci}_{hp}")
                avsO = avsp.tile([P, TCH], BF16, tag="avsO",
                                 name=f"avsO_{tci}_{hp}")
                nc.vector.tensor_copy(avsE[0:D + 1, :], avE[0:D + 1, :])
                nc.vector.tensor_copy(avsO, avO)

                def emit_norm(avsE=avsE, avsO=avsO, dst=aTn[hp], tci=tci,
                              hp=hp):
                    # broadcast each head's den row across 64 partitions via
                    # a K=1 outer-product matmul, reciprocal once, scale.
                    rdn = pmix.tile([P, TCH], F32, tag="rdn",
                                    name=f"rdn_{tci}_{hp}")
                    nc.tensor.matmul(rdn[0:D, :], ones64[D:D + 1, :],
                                     avsE[D:D + 1, :], start=True, stop=True)
                    nc.tensor.matmul(rdn[D:P, :], ones64[0:1, :],
                                     avsO[0:1, :], start=True, stop=True)
                    rdb = rdbp.tile([P, TCH], F32, tag="rdb",
                                    name=f"rdb_{tci}_{hp}")
                    nc.vector.reciprocal_approx_fast(out=rdb, in_=rdn)
                    nc.vector.tensor_mul(dst[0:D, :], avsE[0:D, :],
                                         rdb[0:D, :])
                    nc.vector.tensor_mul(dst[D:P, :], avsO[D:P, :],
                                         rdb[D:P, :])

                state["norm"] = emit_norm

            def emit_outproj(aTn=aTn, tci=tci):
                for ts in range(4):
                    pf = pmix.tile([P, E], F32, tag="pf",
                                   name=f"pf_{tci}_{ts}")
                    for hp2 in range(2):
                        nc.tensor.matmul(pf,
                                         aTn[hp2][:, ts * P:(ts + 1) * P],
                                         wo_sb[hp2],
                                         start=(hp2 == 0), stop=(hp2 == 1))
                    ot = outp.tile([P, E], F32, tag="ot",
                                   name=f"ot_{tci}_{ts}")
                    nc.vector.tensor_copy(ot, pf)
                    row = (tci * 4 + ts) * P
                    nc.sync.dma_start(out=out[row:row + P, :], in_=ot)

            state["out"] = emit_outproj

        with ExitStack() as cp:
            psp = cp.enter_context(
                tc.tile_pool(name="psp", bufs=2, space="PSUM"))
            sig = cp.enter_context(tc.tile_pool(name="sig", bufs=4))
            for tcq in range(NTC):
                proj_group(tcq, psp, sig)
        with ExitStack() as c4:
            pss = c4.enter_context(
                tc.tile_pool(name="pss", bufs=2, space="PSUM"))
            pmix = c4.enter_context(
                tc.tile_pool(name="pmix", bufs=1, space="PSUM"))
            for tcq in range(NTC):
                attention_tci(tcq, pss, pmix)
            state["norm"]()
            state["out"]()

    nc.compile()
    return nc


_NC = None
_LAST_IN_MAPS = None


def _get_nc():
    global _NC
    if _NC is None:
        _NC = _build()
    return _NC


def kernel(**inputs):
    query = np.asarray(inputs["query"], np.float32)
    key_ = np.asarray(inputs["key_"] if "key_" in inputs else inputs["key"],
                      np.float32)
    value = np.asarray(inputs["value"], np.float32)
    Wq = np.asarray(inputs["Wq"], np.float32)
    bq = np.asarray(inputs["bq"], np.float32)
    Wk = np.asarray(inputs["Wk"], np.float32)
    bk = np.asarray(inputs["bk"], np.float32)
    Wv = np.asarray(inputs["Wv"], np.float32)
    bv = np.asarray(inputs["bv"], np.float32)
    Wo = np.asarray(inputs["Wo"], np.float32)
    bo = np.asarray(inputs["bo"], np.float32)
    r = float(np.asarray(inputs["r"]).reshape(-1)[0])

    r_s = 4.0 / (1.0 + np.exp(-np.float64(r)))
    c = np.float32(r_s * r_s / 8.0)

    WqT = Wq.T.astype(BF16NP)
    WkT = Wk.T.astype(BF16NP)
    WoT = Wo.T.astype(BF16NP)
    WvTa = np.concatenate([Wv.T, bv[None, :]], axis=0).astype(BF16NP)

    in_maps = []
    for b in range(B):
        xqT = np.ascontiguousarray(query[b].T).astype(BF16NP)
        xkT = np.ascontiguousarray(key_[b].T).astype(BF16NP)
        xvT = np.concatenate(
            [np.ascontiguousarray(value[b].T), np.ones((1, S), np.float32)],
            axis=0).astype(BF16NP)
        for g in range(HG):
            cols = slice(g * EG, (g + 1) * EG)
            in_maps.append(dict(
                xqT=xqT, xkT=xkT, xvT=xvT,
                wqT=np.ascontiguousarray(WqT[:, cols]),
                wkT=np.ascontiguousarray(WkT[:, cols]),
                wvT=np.ascontiguousarray(WvTa[:, cols]),
                woT=np.ascontiguousarray(WoT[cols, :]),
                bq=np.ascontiguousarray(0.5 * bq[cols, None]),
                bk=np.ascontiguousarray(0.5 * bk[cols, None]),
                cexp=np.array([[c]], np.float32),
            ))

    global _LAST_IN_MAPS
    _LAST_IN_MAPS = in_maps
    res = run_bass_kernel_spmd(_get_nc(), in_maps, core_ids=list(range(NCORES)))
    out = np.empty((B, T, E), np.float32)
    for b in range(B):
        out[b] = res.results[HG * b]["out"]
        for g in range(1, HG):
            out[b] += res.results[HG * b + g]["out"]
        out[b] += bo[None, :]
    return out
